# revision 1
# baseline (speedup 1.0000x reference)
"""Trainium2 8-core kernel for the LaneGCN-style A2A message-passing block.

Strategy (memory-regime):
  - Host: sort edges by destination node (hi), partition destinations across
    8 cores (2500 nodes each), group each core's edges into 20 windows of 128
    destination nodes, pad every window to a common (cross-core) multiple of
    128 edges so one SPMD program serves all cores.
  - Per-node algebra is hoisted out of the edge loop:
      q  = relu(GN(agts @ q_w.T))         (per node)
      QB = q @ B.T,  B = ctx_w1[:,128:256]  (per node, window-sliced tables)
      ctx term uses host-pre-gathered ctx rows (pure data movement)
      scatter:  a.at[hi].add(c @ ctx_w2.T) == (segment_sum c) @ ctx_w2.T
  - Device edge pipeline per 128-edge tile: dist-MLP + GN, y = d@A.T +
    ctx[wi]@C.T + QB[hi] (QB via one-hot matmul), GN+relu, one-hot
    scatter-matmul into per-window PSUM accumulators.
  - Node epilogue: a = agts@agt_w.T + s@ctx_w2.T, GN+relu, lin+GN,
    +residual, relu.
"""

import sys

import numpy as np

if "/opt/trn_rl_repo" not in sys.path:
    sys.path.insert(0, "/opt/trn_rl_repo")

import concourse.bass as bass
import concourse.mybir as mybir
import concourse.tile as tile
from concourse.bass_utils import run_bass_kernel_spmd

N_NODES = 20000
D = 128
NC = 8
NPC = 2500          # nodes per core
NWIN = 20           # windows of 128 dst nodes per core (last window: 68 valid)
NPAD = NWIN * 128   # padded nodes per core (2560)
F32 = mybir.dt.float32


def _apply_drain_patch():
    """This neuronxcc build rejects >2 sem waits on the Tile tail drain
    ("Too many sync wait commands"); split them into single-sem SP waits."""
    from concourse.vector_clock import ScopedClock

    if getattr(tile.TileContext, "_drain_patched", False):
        return

    def _patched(self, tick_clock, wait_clock):
        nc = self.nc
        probe = nc.sync.nop(nofuse=True, hint="drain_wait_probe")
        wait_clock.add_sem_waits(
            probe.ins, ScopedClock({None: tick_clock.global_clock})
        )
        si = probe.ins.sync_info
        waits = list(si.on_wait) if si and si.on_wait else []
        sem_by_id = {h.num: h for h in self.sems.allocated().values()}
        if len(waits) > 2:
            si.on_wait.clear()
            for w in waits:
                h = sem_by_id[w.id]
                nc.sync.wait_ge(h, w.wait_value)
        nc.sync.drain()
        nc.all_engine_barrier()
        popped = nc._tile_sem_poison_stack.pop()
        assert popped is self._sem_poison
        nc.clear_and_free_semaphores(list(self.sems.allocated().values()))
        nc.all_engine_barrier()

    tile.TileContext._drain_and_barrier = _patched
    tile.TileContext._drain_patched = True


def _split_excess_waits(nc, max_waits=1):
    """walrus here rejects instructions with >2 sem-wait commands; hoist the
    excess onto single-wait NoOps inserted just before (same engine)."""
    n = 0
    for f in nc.m.functions:
        for bb in f.blocks:
            out = []
            changed = False
            for ins in bb.instructions:
                si = ins.sync_info
                waits = list(si.on_wait) if si and si.on_wait else []
                if len(waits) > max_waits:
                    keep = waits[-max_waits:]
                    for w in waits[:-max_waits]:
                        nop = mybir.InstNoOp(
                            name=f"I-waitfix-{n}", engine=ins.engine
                        )
                        n += 1
                        nop.sync_info = mybir.SyncInfo(
                            on_wait=[w], on_update=[]
                        )
                        out.append(nop)
                    ins.sync_info = mybir.SyncInfo(
                        on_wait=keep,
                        on_update=list(si.on_update) if si.on_update else [],
                    )
                    changed = True
                out.append(ins)
            if changed:
                bb.instructions = out


def _prep(inputs):
    """Sort/pad edges; build per-core device arrays + shared weight arrays."""
    f = lambda k: np.asarray(inputs[k], dtype=np.float32)
    agts = f("agts")
    ctx = f("ctx")
    agt_ctrs = f("agt_ctrs")
    ctx_ctrs = f("ctx_ctrs")
    hi = np.asarray(inputs["hi"], dtype=np.int64)
    wi = np.asarray(inputs["wi"], dtype=np.int64)

    for g, b in (("dist_g", "dist_beta"), ("q_g", "q_beta"),
                 ("ctx_g", "ctx_beta"), ("norm_g", "norm_beta"),
                 ("lin_g", "lin_beta")):
        assert np.allclose(np.asarray(inputs[g]), 1.0), f"{g} != 1 unsupported"
        assert np.allclose(np.asarray(inputs[b]), 0.0), f"{b} != 0 unsupported"

    order = np.argsort(hi, kind="stable")
    hi_s = hi[order]
    wi_s = wi[order]

    # window boundaries in the sorted edge stream: core c, window k covers
    # dst nodes [c*2500 + 128k, min(c*2500 + 128(k+1), (c+1)*2500))
    node_lo = np.array(
        [c * NPC + k * 128 for c in range(NC) for k in range(NWIN)], np.int64
    )
    node_hi = np.array(
        [min(c * NPC + (k + 1) * 128, (c + 1) * NPC)
         for c in range(NC) for k in range(NWIN)], np.int64
    )
    lo = np.searchsorted(hi_s, node_lo, side="left")
    hicut = np.searchsorted(hi_s, node_hi, side="left")
    cnt = (hicut - lo).reshape(NC, NWIN)

    wk = ((cnt.max(axis=0) + 127) // 128) * 128          # per-window padded len
    wk = np.maximum(wk, 128)
    e_pad = int(wk.sum())
    extra = (-e_pad) % 512
    wk[NWIN - 1] += extra
    e_pad += extra
    woff = np.concatenate([[0], np.cumsum(wk)]).astype(np.int64)
    n_tiles = e_pad // 128

    tile_window = np.empty(n_tiles, np.int64)
    for k in range(NWIN):
        tile_window[woff[k] // 128: woff[k + 1] // 128] = k
    first_tile = (woff[:-1] // 128).astype(np.int64)
    last_tile = (woff[1:] // 128 - 1).astype(np.int64)

    dctr_all = agt_ctrs[hi_s] - ctx_ctrs[wi_s]           # [E, 2]
    ctxg_all = ctx[wi_s]                                  # [E, D]

    per_core = []
    for c in range(NC):
        dctr = np.zeros((e_pad, 2), np.float32)
        ctxg = np.zeros((e_pad, D), np.float32)
        seg = np.full(e_pad, -1.0, np.float32)
        for k in range(NWIN):
            g = c * NWIN + k
            n = cnt[c, k]
            s0, d0 = lo[g], woff[k]
            dctr[d0:d0 + n] = dctr_all[s0:s0 + n]
            ctxg[d0:d0 + n] = ctxg_all[s0:s0 + n]
            seg[d0:d0 + n] = (hi_s[s0:s0 + n] - (c * NPC + k * 128)).astype(
                np.float32
            )
        ag = np.zeros((NPAD, D), np.float32)
        ag[:NPC] = agts[c * NPC:(c + 1) * NPC]
        per_core.append(dict(
            dctr=np.ascontiguousarray(dctr.T),                     # [2, E]
            ctxg=np.ascontiguousarray(ctxg.T),                     # [D, E]
            seg_pm=np.ascontiguousarray(
                seg.reshape(n_tiles, 128).T),                      # [128, T]
            agts_cm=np.ascontiguousarray(ag.T),                    # [D, NPAD]
            agts_nm=np.ascontiguousarray(
                ag.reshape(NWIN, 128, D).transpose(1, 0, 2)
                .reshape(128, NWIN * D)),                          # [128, NWIN*D]
        ))

    w1 = f("dist_w1")       # [D, 2]
    cw1 = f("ctx_w1")       # [D, 3D]
    shared = dict(
        w1T=np.ascontiguousarray(w1.T),                            # [2, D]
        b1=np.ascontiguousarray(f("dist_b1")[:, None]),            # [D, 1]
        w2T=np.ascontiguousarray(f("dist_w2").T),
        AT=np.ascontiguousarray(cw1[:, :D].T),
        BT=np.ascontiguousarray(cw1[:, D:2 * D].T),
        CT=np.ascontiguousarray(cw1[:, 2 * D:].T),
        qwT=np.ascontiguousarray(f("q_w").T),
        xw2T=np.ascontiguousarray(f("ctx_w2").T),
        awT=np.ascontiguousarray(f("agt_w").T),
        lwT=np.ascontiguousarray(f("lin_w").T),
        iota=np.ascontiguousarray(
            np.tile(np.arange(128, dtype=np.float32), (128, 1))),  # [128,128]
        ident=np.eye(128, dtype=np.float32),
    )
    meta = dict(e_pad=e_pad, n_tiles=n_tiles, tile_window=tile_window,
                first_tile=first_tile, last_tile=last_tile)
    return per_core, shared, meta


def _build(meta):
    nc = bass.Bass()
    e_pad = meta["e_pad"]
    n_tiles = meta["n_tiles"]
    tile_window = meta["tile_window"]
    first_tile = meta["first_tile"]
    last_tile = meta["last_tile"]
    n_super = e_pad // 512

    din = {}
    for name, shape in [
        ("dctr", [2, e_pad]), ("ctxg", [D, e_pad]), ("seg_pm", [128, n_tiles]),
        ("agts_cm", [D, NPAD]), ("agts_nm", [128, NWIN * D]),
        ("w1T", [2, D]), ("b1", [D, 1]), ("w2T", [D, D]), ("AT", [D, D]),
        ("BT", [D, D]), ("CT", [D, D]), ("qwT", [D, D]), ("xw2T", [D, D]),
        ("awT", [D, D]), ("lwT", [D, D]), ("iota", [128, 128]),
        ("ident", [128, 128]),
    ]:
        din[name] = nc.dram_tensor(name, shape, F32, kind="ExternalInput")
    out_d = nc.dram_tensor("out", [NPC, D], F32, kind="ExternalOutput")

    with tile.TileContext(nc) as tc:
        with (
            tc.tile_pool(name="consts", bufs=1) as consts,
            tc.tile_pool(name="io", bufs=3) as io,
            tc.tile_pool(name="work", bufs=3) as work,
            tc.tile_pool(name="smalls", bufs=6) as smalls,
            tc.tile_pool(name="ph", bufs=2, space="PSUM") as ph,
            tc.tile_pool(name="pmm", bufs=2, space="PSUM") as pmm,
            tc.tile_pool(name="pwin", bufs=2, space="PSUM") as pwin,
        ):
            cs = {}
            for name in ("w1T", "b1", "w2T", "AT", "BT", "CT", "qwT", "xw2T",
                         "awT", "lwT", "iota", "ident"):
                t = consts.tile(list(din[name].shape), F32, tag=f"c_{name}")
                nc.sync.dma_start(out=t[:], in_=din[name][:])
                cs[name] = t
            seg_sb = consts.tile([128, n_tiles], F32, tag="c_seg")
            nc.sync.dma_start(out=seg_sb[:], in_=din["seg_pm"][:])
            agts_cm = consts.tile([D, NPAD], F32, tag="c_agcm")
            nc.sync.dma_start(out=agts_cm[:], in_=din["agts_cm"][:])
            agts_nm = consts.tile([128, NWIN, D], F32, tag="c_agnm")
            nc.sync.dma_start(
                out=agts_nm[:],
                in_=din["agts_nm"][:].rearrange("p (w d) -> p w d", w=NWIN),
            )
            eps_t = consts.tile([128, 1], F32, tag="c_eps")
            nc.vector.memset(eps_t[:], 1e-5)
            qb_tab = consts.tile([128, NWIN, D], F32, tag="c_qbtab")
            s_tab = consts.tile([128, NWIN, D], F32, tag="c_stab")

            def gn(ps, act, tag):
                """GroupNorm(1,128) over free dim of [128,128] PSUM tile;
                returns normalized (optionally ReLU'd) SBUF tile."""
                st = smalls.tile([128, nc.vector.BN_STATS_DIM], F32,
                                 tag=f"{tag}_st")
                mv = smalls.tile([128, nc.vector.BN_AGGR_DIM], F32,
                                 tag=f"{tag}_mv")
                nc.vector.bn_stats(out=st[:], in_=ps[:])
                nc.vector.bn_aggr(out=mv[:], in_=st[:])
                sd = smalls.tile([128, 1], F32, tag=f"{tag}_sd")
                nc.scalar.activation(
                    out=sd[:], in_=mv[:, 1:2],
                    func=mybir.ActivationFunctionType.Sqrt,
                    bias=eps_t[:], scale=1.0,
                )
                r = smalls.tile([128, 1], F32, tag=f"{tag}_r")
                nc.vector.reciprocal(out=r[:], in_=sd[:])
                b = smalls.tile([128, 1], F32, tag=f"{tag}_b")
                nc.vector.tensor_scalar(
                    out=b[:], in0=mv[:, 0:1], scalar1=r[:], scalar2=-1.0,
                    op0=mybir.AluOpType.mult, op1=mybir.AluOpType.mult,
                )
                o = work.tile([128, 128], F32, tag=f"{tag}_o")
                nc.scalar.activation(
                    out=o[:], in_=ps[:],
                    func=(mybir.ActivationFunctionType.Relu if act
                          else mybir.ActivationFunctionType.Identity),
                    bias=b[:], scale=r[:],
                )
                return o

            def transpose_to_sbuf(src_sb, tag):
                """[128,128] SBUF -> transposed [128,128] SBUF via PE."""
                tp = pmm.tile([128, 128], F32, tag="tp", name=f"tp_{tag}")
                nc.tensor.transpose(out=tp[:], in_=src_sb[:], identity=cs["ident"][:])
                o = work.tile([128, 128], F32, tag=f"{tag}_ts")
                nc.scalar.copy(out=o[:], in_=tp[:])
                return o

            # ---- phase 1: QB table ----
            for t in range(NWIN):
                qp = pmm.tile([128, 128], F32, tag="mm")
                nc.tensor.matmul(
                    out=qp[:], lhsT=agts_cm[:, t * 128:(t + 1) * 128],
                    rhs=cs["qwT"][:], start=True, stop=True,
                )
                q_nm = gn(qp, act=True, tag="gq")
                q_cm = transpose_to_sbuf(q_nm, "q")
                qbp = pmm.tile([128, 128], F32, tag="mm")
                nc.tensor.matmul(out=qbp[:], lhsT=q_cm[:], rhs=cs["BT"][:],
                                 start=True, stop=True)
                nc.vector.tensor_copy(out=qb_tab[:, t, :], in_=qbp[:])

            # ---- phase 2: edge pipeline ----
            win_ps = {}
            for s in range(n_super):
                dctr_t = io.tile([2, 512], F32, tag="dctr")
                nc.sync.dma_start(out=dctr_t[:],
                                  in_=din["dctr"][:, s * 512:(s + 1) * 512])
                ctxg_t = io.tile([D, 512], F32, tag="ctxg")
                nc.sync.dma_start(out=ctxg_t[:],
                                  in_=din["ctxg"][:, s * 512:(s + 1) * 512])
                hp = ph.tile([128, 512], F32, tag="hp")
                nc.tensor.matmul(out=hp[:], lhsT=cs["w1T"][:], rhs=dctr_t[:],
                                 start=True, stop=True)
                h_sb = io.tile([128, 512], F32, tag="hsb")
                nc.scalar.activation(
                    out=h_sb[:], in_=hp[:],
                    func=mybir.ActivationFunctionType.Relu,
                    bias=cs["b1"][:], scale=1.0,
                )
                for i in range(4):
                    gi = s * 4 + i
                    k = int(tile_window[gi])
                    dp = pmm.tile([128, 128], F32, tag="mm")
                    nc.tensor.matmul(
                        out=dp[:], lhsT=h_sb[:, i * 128:(i + 1) * 128],
                        rhs=cs["w2T"][:], start=True, stop=True,
                    )
                    d_sb = gn(dp, act=True, tag="gd")
                    d_cm = transpose_to_sbuf(d_sb, "d")
                    m_sb = work.tile([128, 128], F32, tag="msb")
                    nc.vector.tensor_scalar(
                        out=m_sb[:], in0=cs["iota"][:],
                        scalar1=seg_sb[:, gi:gi + 1], scalar2=None,
                        op0=mybir.AluOpType.is_equal,
                    )
                    m2_sb = transpose_to_sbuf(m_sb, "m2")
                    yp = pmm.tile([128, 128], F32, tag="mm")
                    nc.tensor.matmul(out=yp[:], lhsT=d_cm[:], rhs=cs["AT"][:],
                                     start=True, stop=False)
                    nc.tensor.matmul(
                        out=yp[:], lhsT=ctxg_t[:, i * 128:(i + 1) * 128],
                        rhs=cs["CT"][:], start=False, stop=False,
                    )
                    nc.tensor.matmul(
                        out=yp[:], lhsT=m2_sb[:], rhs=qb_tab[:, k, :],
                        start=False, stop=True,
                    )
                    c_sb = gn(yp, act=True, tag="gc")
                    if gi == first_tile[k]:
                        win_ps[k] = pwin.tile([128, 128], F32, tag="swin", name=f"swin{k}")
                    nc.tensor.matmul(
                        out=win_ps[k][:], lhsT=c_sb[:], rhs=m_sb[:],
                        start=(gi == first_tile[k]), stop=(gi == last_tile[k]),
                    )
                    if gi == last_tile[k]:
                        nc.vector.tensor_copy(out=s_tab[:, k, :],
                                              in_=win_ps[k][:])
                        del win_ps[k]

            # ---- phase 3: node epilogue ----
            for t in range(NWIN):
                ap = pmm.tile([128, 128], F32, tag="mm")
                nc.tensor.matmul(
                    out=ap[:], lhsT=agts_cm[:, t * 128:(t + 1) * 128],
                    rhs=cs["awT"][:], start=True, stop=False,
                )
                nc.tensor.matmul(out=ap[:], lhsT=s_tab[:, t, :],
                                 rhs=cs["xw2T"][:], start=False, stop=True)
                a1 = gn(ap, act=True, tag="ga1")
                a1_cm = transpose_to_sbuf(a1, "a1")
                a2p = pmm.tile([128, 128], F32, tag="mm")
                nc.tensor.matmul(out=a2p[:], lhsT=a1_cm[:], rhs=cs["lwT"][:],
                                 start=True, stop=True)
                a2n = gn(a2p, act=False, tag="ga2")
                o_sb = work.tile([128, 128], F32, tag="osb")
                nc.vector.tensor_tensor(
                    out=o_sb[:], in0=a2n[:], in1=agts_nm[:, t, :],
                    op=mybir.AluOpType.add,
                )
                o2 = work.tile([128, 128], F32, tag="o2")
                nc.scalar.activation(
                    out=o2[:], in_=o_sb[:],
                    func=mybir.ActivationFunctionType.Relu, bias=0.0, scale=1.0,
                )
                nrow = 128 if t < NWIN - 1 else NPC - (NWIN - 1) * 128
                nc.sync.dma_start(
                    out=out_d[t * 128:t * 128 + nrow, :], in_=o2[:nrow, :]
                )
    _split_excess_waits(nc)
    return nc


def kernel(**inputs):
    _apply_drain_patch()
    per_core, shared, meta = _prep(inputs)
    nc = _build(meta)
    in_maps = [{**per_core[c], **shared} for c in range(NC)]
    res = run_bass_kernel_spmd(nc, in_maps, core_ids=list(range(NC)))
    out = np.concatenate([res.results[c]["out"] for c in range(NC)], axis=0)
    return out.astype(np.float32)



# revision 12
# speedup vs baseline: 5.9342x; 5.9342x over previous
"""Trainium2 8-core kernel for the LaneGCN-style A2A message-passing block.

Strategy (memory-regime):
  - Host: sort edges by destination node (hi), partition destinations across
    8 cores (2500 nodes each), group each core's edges into 20 windows of 128
    destination nodes, pad every window to a common (cross-core) multiple of
    128 edges so one SPMD program serves all cores.
  - All GroupNorms are mean-free by construction: every Linear that feeds a
    GN gets its weight matrix column-mean-subtracted on the host
    (w_c = w - mean_over_out_rows), so the GN mean is exactly 0 and the
    device only computes the variance (bn_stats) and folds the 1/std scale
    into the ReLU evacuation (Act: out = relu(x * scale)).
  - Per-node algebra hoisted out of the edge loop (QB table, ctx gathered on
    host into a [D, E] bf16 stream, scatter via one-hot matmul into
    per-window PSUM accumulators).
  - Edge pipeline runs on 512-edge supertiles: single-bank [128, 512] PSUM
    accumulators for the dist-MLP (dp) and the fused y (yp), one batched
    bn_stats / bn_aggr / sqrt / reciprocal per supertile per GN, batched
    transpose evacuations (DVE), bf16 matmuls (4x PE speedup vs fp32).
"""

import sys

import numpy as np

if "/opt/trn_rl_repo" not in sys.path:
    sys.path.insert(0, "/opt/trn_rl_repo")

import concourse.bass as bass
import concourse.mybir as mybir
import concourse.tile as tile
from concourse.bass_utils import run_bass_kernel_spmd

N_NODES = 20000
D = 128
NC = 8
NPC = 2500          # nodes per core
NWIN = 20           # windows of 128 dst nodes per core (last window: 68 valid)
NPAD = NWIN * 128   # padded nodes per core (2560)
F32 = mybir.dt.float32
F32R = mybir.dt.float32r
BF16 = mybir.dt.bfloat16
NP_BF16 = mybir.dt.np(BF16)


def _apply_drain_patch():
    """This neuronxcc build rejects >2 sem waits on the Tile tail drain
    ("Too many sync wait commands"); split them into single-sem SP waits."""
    from concourse.vector_clock import ScopedClock

    if getattr(tile.TileContext, "_drain_patched", False):
        return

    def _patched(self, tick_clock, wait_clock):
        nc = self.nc
        probe = nc.sync.nop(nofuse=True, hint="drain_wait_probe")
        wait_clock.add_sem_waits(
            probe.ins, ScopedClock({None: tick_clock.global_clock})
        )
        si = probe.ins.sync_info
        waits = list(si.on_wait) if si and si.on_wait else []
        sem_by_id = {h.num: h for h in self.sems.allocated().values()}
        if len(waits) > 2:
            si.on_wait.clear()
            for w in waits:
                h = sem_by_id[w.id]
                nc.sync.wait_ge(h, w.wait_value)
        nc.sync.drain()
        nc.all_engine_barrier()
        popped = nc._tile_sem_poison_stack.pop()
        assert popped is self._sem_poison
        nc.clear_and_free_semaphores(list(self.sems.allocated().values()))
        nc.all_engine_barrier()

    tile.TileContext._drain_and_barrier = _patched
    tile.TileContext._drain_patched = True


def _split_excess_waits(nc, max_waits=1):
    """walrus here rejects instructions with >2 sem-wait commands; hoist the
    excess onto single-wait NoOps inserted just before (same engine)."""
    n = 0
    for f in nc.m.functions:
        for bb in f.blocks:
            out = []
            changed = False
            for ins in bb.instructions:
                si = ins.sync_info
                waits = list(si.on_wait) if si and si.on_wait else []
                if len(waits) > max_waits:
                    keep = waits[-max_waits:]
                    for w in waits[:-max_waits]:
                        nop = mybir.InstNoOp(
                            name=f"I-waitfix-{n}", engine=ins.engine
                        )
                        n += 1
                        nop.sync_info = mybir.SyncInfo(
                            on_wait=[w], on_update=[]
                        )
                        out.append(nop)
                    ins.sync_info = mybir.SyncInfo(
                        on_wait=keep,
                        on_update=list(si.on_update) if si.on_update else [],
                    )
                    changed = True
                out.append(ins)
            if changed:
                bb.instructions = out
    return nc


def _center(w):
    # w maps x -> x @ w.T; subtracting the mean over output rows makes the
    # output exactly zero-mean across features.
    return w - w.mean(axis=0, keepdims=True)


def _prep(inputs):
    """Sort/pad edges; build per-core device arrays + shared weight arrays."""
    f = lambda k: np.asarray(inputs[k], dtype=np.float32)
    agts = f("agts")
    ctx = f("ctx")
    agt_ctrs = f("agt_ctrs")
    ctx_ctrs = f("ctx_ctrs")
    hi = np.asarray(inputs["hi"], dtype=np.int64)
    wi = np.asarray(inputs["wi"], dtype=np.int64)

    for g, b in (("dist_g", "dist_beta"), ("q_g", "q_beta"),
                 ("ctx_g", "ctx_beta"), ("norm_g", "norm_beta"),
                 ("lin_g", "lin_beta")):
        assert np.allclose(np.asarray(inputs[g]), 1.0), f"{g} != 1 unsupported"
        assert np.allclose(np.asarray(inputs[b]), 0.0), f"{b} != 0 unsupported"

    order = np.argsort(hi, kind="stable")
    hi_s = hi[order]
    wi_s = wi[order]

    # window boundaries in the sorted edge stream: core c, window k covers
    # dst nodes [c*2500 + 128k, min(c*2500 + 128(k+1), (c+1)*2500))
    node_lo = np.array(
        [c * NPC + k * 128 for c in range(NC) for k in range(NWIN)], np.int64
    )
    node_hi = np.array(
        [min(c * NPC + (k + 1) * 128, (c + 1) * NPC)
         for c in range(NC) for k in range(NWIN)], np.int64
    )
    lo = np.searchsorted(hi_s, node_lo, side="left")
    hicut = np.searchsorted(hi_s, node_hi, side="left")
    cnt = (hicut - lo).reshape(NC, NWIN)

    wk = ((cnt.max(axis=0) + 127) // 128) * 128          # per-window padded len
    wk = np.maximum(wk, 128)
    e_pad = int(wk.sum())
    extra = (-e_pad) % 512
    wk[NWIN - 1] += extra
    e_pad += extra
    woff = np.concatenate([[0], np.cumsum(wk)]).astype(np.int64)
    n_tiles = e_pad // 128

    tile_window = np.empty(n_tiles, np.int64)
    for k in range(NWIN):
        tile_window[woff[k] // 128: woff[k + 1] // 128] = k
    first_tile = (woff[:-1] // 128).astype(np.int64)
    last_tile = (woff[1:] // 128 - 1).astype(np.int64)

    dctr_all = agt_ctrs[hi_s] - ctx_ctrs[wi_s]           # [E, 2]
    ctx_bf = ctx.astype(NP_BF16)
    ctxg_all = ctx_bf[wi_s]                               # [E, D] bf16

    per_core = []
    for c in range(NC):
        dctr = np.zeros((e_pad, 2), np.float32)
        ctxg = np.zeros((e_pad, D), NP_BF16)
        seg = np.full(e_pad, -1.0, np.float32)
        for k in range(NWIN):
            g = c * NWIN + k
            n = cnt[c, k]
            s0, d0 = lo[g], woff[k]
            dctr[d0:d0 + n] = dctr_all[s0:s0 + n]
            ctxg[d0:d0 + n] = ctxg_all[s0:s0 + n]
            seg[d0:d0 + n] = (hi_s[s0:s0 + n] - (c * NPC + k * 128)).astype(
                np.float32
            )
        ag = np.zeros((NPAD, D), np.float32)
        ag[:NPC] = agts[c * NPC:(c + 1) * NPC]
        per_core.append(dict(
            dctr=np.ascontiguousarray(dctr.T),                     # [2, E]
            ctxg=np.ascontiguousarray(ctxg.T),                     # [D, E] bf16
            seg_pm=np.ascontiguousarray(
                seg.reshape(n_tiles, 128).T),                      # [128, T]
            agts_cm=np.ascontiguousarray(ag.T),                    # [D, NPAD]
            agts_nm=np.ascontiguousarray(
                ag.reshape(NWIN, 128, D).transpose(1, 0, 2)
                .reshape(128, NWIN * D)),                          # [128, NWIN*D]
        ))

    w1 = f("dist_w1")       # [D, 2]
    cw1 = f("ctx_w1")       # [D, 3D]
    bfT = lambda w: np.ascontiguousarray(w.T).astype(NP_BF16)
    f32T = lambda w: np.ascontiguousarray(w.T)
    shared = dict(
        w1T=np.ascontiguousarray(w1.T),                            # [2, D] f32r
        b1=np.ascontiguousarray(f("dist_b1")[:, None]),            # [D, 1]
        w2T=bfT(_center(f("dist_w2"))),
        AT=bfT(_center(cw1[:, :D])),
        BT=bfT(_center(cw1[:, D:2 * D])),
        CT=bfT(_center(cw1[:, 2 * D:])),
        qwT=f32T(_center(f("q_w"))),
        xw2T=f32T(_center(f("ctx_w2"))),
        awT=f32T(_center(f("agt_w"))),
        lwT=bfT(_center(f("lin_w"))),
        iota=np.ascontiguousarray(
            np.tile(np.arange(128, dtype=np.float32), (128, 1))),  # [128,128]
        ident_bf=np.eye(128, dtype=NP_BF16),
        ident=np.eye(128, dtype=np.float32),
    )
    meta = dict(e_pad=e_pad, n_tiles=n_tiles, tile_window=tile_window,
                first_tile=first_tile, last_tile=last_tile)
    return per_core, shared, meta


def _build(meta):
    nc = bass.Bass()
    e_pad = meta["e_pad"]
    n_tiles = meta["n_tiles"]
    tile_window = meta["tile_window"]
    first_tile = meta["first_tile"]
    last_tile = meta["last_tile"]
    n_super = e_pad // 512

    din = {}
    for name, shape, dt in [
        ("dctr", [2, e_pad], F32R), ("ctxg", [D, e_pad], BF16),
        ("seg_pm", [128, n_tiles], F32),
        ("agts_cm", [D, NPAD], F32), ("agts_nm", [128, NWIN * D], F32),
        ("w1T", [2, D], F32R), ("b1", [D, 1], F32), ("w2T", [D, D], BF16),
        ("AT", [D, D], BF16), ("BT", [D, D], BF16), ("CT", [D, D], BF16),
        ("qwT", [D, D], F32), ("xw2T", [D, D], F32),
        ("awT", [D, D], F32), ("lwT", [D, D], BF16),
        ("iota", [128, 128], F32), ("ident_bf", [128, 128], BF16),
        ("ident", [128, 128], F32),
    ]:
        din[name] = nc.dram_tensor(name, shape, dt, kind="ExternalInput")
    out_d = nc.dram_tensor("out", [NPC, D], F32, kind="ExternalOutput")

    RELU = mybir.ActivationFunctionType.Relu
    IDENT = mybir.ActivationFunctionType.Identity
    SQRT = mybir.ActivationFunctionType.Sqrt

    with tile.TileContext(nc) as tc:
        with (
            tc.tile_pool(name="consts", bufs=1) as consts,
            tc.tile_pool(name="io", bufs=3) as io,
            tc.tile_pool(name="work", bufs=3) as work,
            tc.tile_pool(name="smalls", bufs=4) as smalls,
            tc.tile_pool(name="ph", bufs=1, space="PSUM") as ph,
            tc.tile_pool(name="pdp", bufs=2, space="PSUM") as pdp,
            tc.tile_pool(name="pyp", bufs=2, space="PSUM") as pyp,
            tc.tile_pool(name="ptr", bufs=2, space="PSUM") as ptr,
            tc.tile_pool(name="pwin", bufs=1, space="PSUM") as pwin,
        ):
            cs = {}
            for name in ("w1T", "b1", "w2T", "AT", "BT", "CT", "qwT", "xw2T",
                         "awT", "lwT", "iota", "ident_bf", "ident"):
                t = consts.tile(list(din[name].shape), din[name].dtype,
                                tag=f"c_{name}")
                nc.sync.dma_start(out=t[:], in_=din[name][:])
                cs[name] = t
            seg_sb = consts.tile([128, n_tiles], F32, tag="c_seg")
            nc.sync.dma_start(out=seg_sb[:], in_=din["seg_pm"][:])
            agts_cm = consts.tile([D, NPAD], F32, tag="c_agcm")
            nc.sync.dma_start(out=agts_cm[:], in_=din["agts_cm"][:])
            agts_nm = consts.tile([128, NWIN, D], F32, tag="c_agnm")
            nc.sync.dma_start(
                out=agts_nm[:],
                in_=din["agts_nm"][:].rearrange("p (w d) -> p w d", w=NWIN),
            )
            eps_t = consts.tile([128, 1], F32, tag="c_eps")
            nc.vector.memset(eps_t[:], 1e-5)
            qb_tab = consts.tile([128, NWIN, D], BF16, tag="c_qbtab")
            s_tab = consts.tile([128, NWIN, D], F32, tag="c_stab")

            def gn_scale(ps, ng, tag):
                """GN scale for `ng` row-major [128,128] tiles packed along
                the free dim of one PSUM tile. Mean is exactly 0 (host-
                centered weights); returns r [128, ng] = rsqrt(var+eps).
                bn_stats/bn_aggr run per 128-wide group (HW limitation); the
                sqrt and reciprocal are batched across groups.
                """
                st = smalls.tile([128, ng, nc.vector.BN_STATS_DIM], F32,
                                 tag=f"{tag}_st")
                mv = smalls.tile([128, ng, nc.vector.BN_AGGR_DIM], F32,
                                 tag=f"{tag}_mv")
                for g in range(ng):
                    nc.vector.bn_stats(out=st[:, g, :],
                                       in_=ps[:, g * 128:(g + 1) * 128])
                    nc.vector.bn_aggr(out=mv[:, g, :], in_=st[:, g, :])
                sd = smalls.tile([128, ng], F32, tag=f"{tag}_sd")
                nc.scalar.activation(
                    out=sd[:], in_=mv[:, :, 1],
                    func=SQRT, bias=eps_t[:], scale=1.0,
                )
                r = smalls.tile([128, ng], F32, tag=f"{tag}_r")
                nc.vector.reciprocal(out=r[:], in_=sd[:])
                return r

            def gn_128(ps, act, tag):
                """Single-tile GN (phases 1/3): normalized (+ReLU) f32 tile."""
                r = gn_scale(ps[:], 1, tag)
                o = work.tile([128, 128], F32, tag=f"{tag}_o")
                nc.scalar.activation(
                    out=o[:], in_=ps[:], func=(RELU if act else IDENT),
                    bias=0.0, scale=r[:, 0:1],
                )
                return o

            # ---- phase 1: QB table (q = relu(GN(agts@qw.T)); QB = q@B.T) ----
            for t in range(NWIN):
                qp = pdp.tile([128, 128], F32, tag="dp", name=f"qp{t}")
                nc.tensor.matmul(
                    out=qp[:], lhsT=agts_cm[:, t * 128:(t + 1) * 128],
                    rhs=cs["qwT"][:], start=True, stop=True,
                )
                rq = gn_scale(qp[:], 1, "gq")
                q_nm = work.tile([128, 128], BF16, tag="qnm")
                nc.scalar.activation(out=q_nm[:], in_=qp[:], func=RELU,
                                     bias=0.0, scale=rq[:, 0:1])
                qtp = ptr.tile([128, 128], BF16, tag="tr", name=f"tp_q{t}")
                nc.tensor.transpose(out=qtp[:], in_=q_nm[:],
                                    identity=cs["ident_bf"][:])
                q_cm = work.tile([128, 128], BF16, tag="qcm")
                nc.vector.tensor_copy(out=q_cm[:], in_=qtp[:])
                qbp = pyp.tile([128, 128], F32, tag="yp", name=f"qbp{t}")
                nc.tensor.matmul(out=qbp[:], lhsT=q_cm[:], rhs=cs["BT"][:],
                                 start=True, stop=True)
                nc.vector.tensor_copy(out=qb_tab[:, t, :], in_=qbp[:])

            # ---- phase 2: edge pipeline over 512-edge supertiles ----
            win_ps = {}
            for s in range(n_super):
                dctr_t = io.tile([2, 512], F32R, tag="dctr")
                nc.sync.dma_start(out=dctr_t[:],
                                  in_=din["dctr"][:, s * 512:(s + 1) * 512])
                ctxg_t = io.tile([D, 512], BF16, tag="ctxg")
                nc.sync.dma_start(out=ctxg_t[:],
                                  in_=din["ctxg"][:, s * 512:(s + 1) * 512])
                # h = relu(w1 @ dctr + b1): [d, e] feature-major
                hp = ph.tile([128, 512], F32, tag="hp")
                nc.tensor.matmul(out=hp[:], lhsT=cs["w1T"][:], rhs=dctr_t[:],
                                 start=True, stop=True)
                h_sb = work.tile([128, 512], BF16, tag="hsb")
                nc.scalar.activation(out=h_sb[:], in_=hp[:], func=RELU,
                                     bias=cs["b1"][:], scale=1.0)
                # dist MLP: dp[i] = (h_i).T @ w2c.T  -> [e, d] rows per tile
                dp = pdp.tile([128, 512], F32, tag="dp")
                for i in range(4):
                    nc.tensor.matmul(
                        out=dp[:, i * 128:(i + 1) * 128],
                        lhsT=h_sb[:, i * 128:(i + 1) * 128],
                        rhs=cs["w2T"][:], start=True, stop=True,
                    )
                r_d = gn_scale(dp[:], 4, "gd")
                # per-tile scaled relu evac (d2 = r_d * relu(dp))
                d_sb = work.tile([128, 512], BF16, tag="dsb")
                for i in range(4):
                    nc.scalar.activation(
                        out=d_sb[:, i * 128:(i + 1) * 128],
                        in_=dp[:, i * 128:(i + 1) * 128],
                        func=RELU, bias=0.0, scale=r_d[:, i:i + 1],
                    )
                # transposes: d2 -> [d, e]; masks -> m2 [node, e]
                tr = ptr.tile([128, 1024], BF16, tag="tr")
                trd = tr[:, 0:512]
                trm = tr[:, 512:1024]
                m_raw = work.tile([128, 4, 128], BF16, tag="mraw")
                for i in range(4):
                    gi = s * 4 + i
                    nc.vector.tensor_scalar(
                        out=m_raw[:, i, :], in0=cs["iota"][:],
                        scalar1=seg_sb[:, gi:gi + 1], scalar2=None,
                        op0=mybir.AluOpType.is_equal,
                    )
                    nc.tensor.transpose(
                        out=trd[:, i * 128:(i + 1) * 128],
                        in_=d_sb[:, i * 128:(i + 1) * 128],
                        identity=cs["ident_bf"][:],
                    )
                    nc.tensor.transpose(
                        out=trm[:, i * 128:(i + 1) * 128],
                        in_=m_raw[:, i, :],
                        identity=cs["ident_bf"][:],
                    )
                d_cm = work.tile([128, 512], BF16, tag="dcm")
                nc.vector.tensor_copy(out=d_cm[:], in_=trd)
                m2_sb = work.tile([128, 512], BF16, tag="m2sb")
                nc.vector.tensor_copy(out=m2_sb[:], in_=trm)
                # y = d2@A.T + ctx@C.T + QB[hi]  (QB via one-hot matmul)
                yp = pyp.tile([128, 512], F32, tag="yp")
                for i in range(4):
                    gi = s * 4 + i
                    k = int(tile_window[gi])
                    sl = slice(i * 128, (i + 1) * 128)
                    nc.tensor.matmul(out=yp[:, sl], lhsT=d_cm[:, sl],
                                     rhs=cs["AT"][:], start=True, stop=False)
                    nc.tensor.matmul(out=yp[:, sl], lhsT=ctxg_t[:, sl],
                                     rhs=cs["CT"][:], start=False, stop=False)
                    nc.tensor.matmul(out=yp[:, sl], lhsT=m2_sb[:, sl],
                                     rhs=qb_tab[:, k, :], start=False,
                                     stop=True)
                r_y = gn_scale(yp[:], 4, "gy")
                c_sb = work.tile([128, 512], BF16, tag="csb")
                for i in range(4):
                    nc.scalar.activation(
                        out=c_sb[:, i * 128:(i + 1) * 128],
                        in_=yp[:, i * 128:(i + 1) * 128],
                        func=RELU, bias=0.0, scale=r_y[:, i:i + 1],
                    )
                # scatter: win[k] += c_i.T @ m_raw_i   ([d, node] accumulators)
                for i in range(4):
                    gi = s * 4 + i
                    k = int(tile_window[gi])
                    if gi == first_tile[k]:
                        win_ps[k] = pwin.tile([128, 128], F32, tag="swin",
                                              name=f"swin{k}")
                    nc.tensor.matmul(
                        out=win_ps[k][:],
                        lhsT=c_sb[:, i * 128:(i + 1) * 128],
                        rhs=m_raw[:, i, :],
                        start=(gi == first_tile[k]),
                        stop=(gi == last_tile[k]),
                    )
                    if gi == last_tile[k]:
                        nc.vector.tensor_copy(out=s_tab[:, k, :],
                                              in_=win_ps[k][:])
                        del win_ps[k]

            # ---- phase 3: node epilogue ----
            for t in range(NWIN):
                ap = pdp.tile([128, 128], F32, tag="dp", name=f"ap{t}")
                nc.tensor.matmul(
                    out=ap[:], lhsT=agts_cm[:, t * 128:(t + 1) * 128],
                    rhs=cs["awT"][:], start=True, stop=False,
                )
                nc.tensor.matmul(out=ap[:], lhsT=s_tab[:, t, :],
                                 rhs=cs["xw2T"][:], start=False, stop=True)
                ra = gn_scale(ap[:], 1, "ga1")
                a1 = work.tile([128, 128], BF16, tag="a1")
                nc.scalar.activation(out=a1[:], in_=ap[:], func=RELU,
                                     bias=0.0, scale=ra[:, 0:1])
                atp = ptr.tile([128, 128], BF16, tag="tr", name=f"tp_a{t}")
                nc.tensor.transpose(out=atp[:], in_=a1[:],
                                    identity=cs["ident_bf"][:])
                a1_cm = work.tile([128, 128], BF16, tag="a1cm")
                nc.vector.tensor_copy(out=a1_cm[:], in_=atp[:])
                a2p = pyp.tile([128, 128], F32, tag="yp", name=f"a2p{t}")
                nc.tensor.matmul(out=a2p[:], lhsT=a1_cm[:], rhs=cs["lwT"][:],
                                 start=True, stop=True)
                a2n = gn_128(a2p, act=False, tag="ga2")
                o_sb = work.tile([128, 128], F32, tag="osb")
                nc.vector.tensor_tensor(
                    out=o_sb[:], in0=a2n[:], in1=agts_nm[:, t, :],
                    op=mybir.AluOpType.add,
                )
                o2 = work.tile([128, 128], F32, tag="o2")
                nc.scalar.activation(out=o2[:], in_=o_sb[:], func=RELU,
                                     bias=0.0, scale=1.0)
                nrow = 128 if t < NWIN - 1 else NPC - (NWIN - 1) * 128
                nc.sync.dma_start(
                    out=out_d[t * 128:t * 128 + nrow, :], in_=o2[:nrow, :]
                )
    _split_excess_waits(nc)
    return nc


def kernel(**inputs):
    _apply_drain_patch()
    per_core, shared, meta = _prep(inputs)
    nc = _build(meta)
    in_maps = [{**per_core[c], **shared} for c in range(NC)]
    res = run_bass_kernel_spmd(nc, in_maps, core_ids=list(range(NC)))
    out = np.concatenate([res.results[c]["out"] for c in range(NC)], axis=0)
    return out.astype(np.float32)
